# revision 32
# baseline (speedup 1.0000x reference)
"""AutoCov1D Trainium2 kernel (8 NeuronCores, data-parallel over batch).

Math: for window n (stride 8, width 64), with X1 = X[:, :-64], X2 = X[:, 64:]:
  p1 = einsum('bnw,wdc', X1win, Wgt); p2 likewise with X2win
  out = mean_d(p1c * p2c) + bias   (p*c centered over d)

Exact simplifications:
  1. Centering over d is linear in the weight, so pre-center the weight:
     Wtil = (W - mean_d W) / sqrt(D); then no mean terms remain.
  2. X2 windows are X1 windows shifted by 8 window indices (64 = 8*stride),
     so ONE projection P[b,m,:] = sum_w X[b, 8m+w] * Wtil[w,:] over m=0..504
     serves both operands:  out[b,n,c] = sum_d P[b,n,d,c]*P[b,n+8,d,c] + bias.

Implementation notes:
  - X pre-decimated on host: xdt[w, b, j] = X[b, 8j + w] -> contiguous
    projection matmul rhs, 8x less X DMA than a shifted-rows layout; X is
    duplicated into partitions 64-127 so each cb pair of projections runs
    CONCURRENTLY as K=64 PE row tiles at row positions 0 and 64.
  - bf16 products (DVE 2x tensor_tensor, one op per dq pair); latent-dim
    reduction via K=128 -> M=32 selector matmuls at 4 concurrent PE
    column tiles, accumulating the 8 dq quads in fp32 PSUM.
  - Selector matmuls interleave into the same batch row's projection
    stream once their dq pair's products land, avoiding a long serial
    selector tail.
  - PSUM evacuation (fp32->bf16, 1x rate on either engine) splits ~4:1
    ACT:DVE to balance the two engines; DVE additionally runs the
    products and the bias add, ACT only evacuates.
  - PE warmup matmuls on a zeroed scratch tile keep the tensor engine
    busy (p-state) while input DMAs land; DMA triggers are spread across
    the ACT/SP/GPSIMD queues so they issue in parallel at startup.
"""

import sys

import numpy as np

if "/opt/trn_rl_repo" not in sys.path:
    sys.path.insert(0, "/opt/trn_rl_repo")

_B, _T, _W, _D, _C = 32, 4096, 64, 32, 128
_NCORES = 8
_BSH = _B // _NCORES  # 4
_M = 505  # projection windows per batch row
_N = 497  # output windows per batch row
_S = 8  # stride, also window shift in m-space
_MM = 506  # projection matmul free dim (even; last col pad)
_JW = 512  # padded xdt column count
_WSCALE = 1.0  # weight prescale (1 = none)

_NC_CACHE = None


def _build_nc():
    import concourse.bass as bass
    import concourse.tile as tile
    from concourse import bacc, mybir
    from contextlib import ExitStack

    f32 = mybir.dt.float32
    bf16 = mybir.dt.bfloat16

    nc = bacc.Bacc(None, target_bir_lowering=False)
    # xdt[w, b, j] = X[b, 8j + w] (zero past the end), duplicated into
    # rows 64-127 for the row-tiled projection matmuls
    xdt = nc.declare_dram_parameter("xdt", [2 * _W, _BSH, _JW], bf16, isOutput=False)
    # wt[p, dq, cp, dd*32+cc]: p<64 -> Wtil[p, 4*dq+dd, 64*cp+cc+0]
    # row-half layout, p>=64 -> Wtil[p-64, 4*dq+dd, 64*cp+cc+32]
    wt = nc.declare_dram_parameter("wt", [2 * _W, 8, 2, _C], bf16, isOutput=False)
    sel = nc.declare_dram_parameter("sel", [_C, 32], bf16, isOutput=False)
    bias = nc.declare_dram_parameter("bias", [_C, 1], f32, isOutput=False)
    out = nc.declare_dram_parameter("out", [_BSH, _C, _N], f32, isOutput=True)

    with ExitStack() as ctx:
        tc = ctx.enter_context(tile.TileContext(nc))
        singles = ctx.enter_context(tc.tile_pool(name="singles", bufs=1))
        psp = ctx.enter_context(tc.tile_pool(name="psp", bufs=2, space="PSUM"))
        covp = ctx.enter_context(tc.tile_pool(name="covp", bufs=2, space="PSUM"))
        evacp = ctx.enter_context(tc.tile_pool(name="evacp", bufs=4))
        prodp = ctx.enter_context(tc.tile_pool(name="prodp", bufs=4))
        outp = ctx.enter_context(tc.tile_pool(name="outp", bufs=2))

        # PE warmup scratch (zeroed; matmuls on it ramp the p-state while
        # the real input DMAs land)
        warm = singles.tile([_W, _JW], bf16)
        nc.gpsimd.memset(warm[:, :], 0)

        # first dq-slice of weights + first batch row of X land first so
        # real work starts early
        wt_sb = [
            singles.tile([2 * _W, 2, _C], bf16, name=f"wt{dq}") for dq in range(8)
        ]
        # X duplicated into partitions 64-127 so row-tiled projection
        # matmuls at PE row position 64 can stream it
        xdt_sb = [
            singles.tile([2 * _W, _JW], bf16, name=f"xd{b}") for b in range(_BSH)
        ]
        nc.scalar.dma_start(out=xdt_sb[0], in_=xdt[:, 0, :])
        nc.sync.dma_start(out=wt_sb[0], in_=wt[:, 0, :, :])
        sel_sb = singles.tile([_C, 32], bf16)
        nc.gpsimd.dma_start(out=sel_sb, in_=sel[:, :])
        bias_sb = singles.tile([_C, 1], f32)
        nc.gpsimd.dma_start(out=bias_sb, in_=bias[:, :])
        for dq in range(1, 8):
            nc.sync.dma_start(out=wt_sb[dq], in_=wt[:, dq, :, :])
        for b in range(1, _BSH):
            nc.sync.dma_start(out=xdt_sb[b], in_=xdt[:, b, :])

        # warmup matmuls (results discarded)
        wps = covp.tile([_C, _JW], f32, tag="cov")
        for i in range(8):
            nc.tensor.matmul(
                wps[:, 0:_MM],
                lhsT=warm[:, 0:_C],
                rhs=warm[:, 0:_MM],
                start=True,
                stop=True,
            )

        def sel_dq(cov, pr, dq):
            # latent-dim reduction: K=128 -> M=32 selector matmuls, fp32
            # PSUM accumulation over the 8 dq quads per channel block
            for cb in range(4):
                nc.tensor.matmul(
                    cov[32 * cb : 32 * cb + 32, 0:_N],
                    lhsT=sel_sb[:, :],
                    rhs=pr[:, 4 * (dq % 2) + cb, 0:_N],
                    start=(dq == 0),
                    stop=(dq == 7),
                    tile_position=(0, 32 * cb),
                )

        ev_i = 0
        for b in range(_BSH):
            cov = covp.tile([_C, _JW], f32, tag="cov")
            pr_dqs = [None] * 4
            ev = None
            for dq in range(8):
                if dq % 2 == 0:
                    ev = evacp.tile([_C, 8, _JW], bf16, name=f"ev{b}_{dq}", tag="ev")
                    pr_dqs[dq // 2] = prodp.tile(
                        [_C, 8, _JW], bf16, name=f"pr{b}_{dq}", tag="pr"
                    )
                for cp in range(2):
                    # two K=64 projections run concurrently as PE row
                    # tiles at row positions 0 and 64
                    for j in range(2):
                        g = 4 * (dq % 2) + 2 * cp + j
                        if g % 3 == 0:
                            ps = psp.tile([_C, 3, 512], f32, name="ps", tag="ps")
                            ps_base = g
                        nc.tensor.matmul(
                            ps[:, g - ps_base, 0:_MM],
                            lhsT=wt_sb[dq][j * _W : (j + 1) * _W, cp, :],
                            rhs=xdt_sb[b][j * _W : (j + 1) * _W, 0:_MM],
                            start=True,
                            stop=True,
                            tile_position=(j * _W, 0),
                        )
                        # evacuate fp32 PSUM -> bf16 SBUF (1x rate on
                        # either engine) once the tile's slots are full;
                        # a few ops go to DVE to balance the engines
                        if g - ps_base == 2 or g == 7:
                            nsl = g - ps_base + 1
                            dst = ev[:, ps_base : ps_base + nsl, 0:_MM]
                            if ev_i % 16 in (4, 9, 14):
                                nc.vector.tensor_copy(dst, ps[:, 0:nsl, 0:_MM])
                            else:
                                nc.scalar.copy(out=dst, in_=ps[:, 0:nsl, 0:_MM])
                            ev_i += 1
                # one shifted-product op per dq pair (8 cb slices)
                if dq % 2 == 1:
                    nc.vector.tensor_mul(
                        pr_dqs[dq // 2][:, :, 0:_N],
                        ev[:, :, 0:_N],
                        ev[:, :, _S : _S + _N],
                    )
                # selector for pair p is ready after product of dq=2p+1;
                # emit with 2-dq slack so the PE never stalls on the
                # evac+product chain
                if dq in (5, 7):
                    p = (dq - 5) // 2
                    for dqs in (2 * p, 2 * p + 1):
                        sel_dq(cov, pr_dqs[p], dqs)
            for p in (2, 3):
                for dqs in (2 * p, 2 * p + 1):
                    sel_dq(cov, pr_dqs[p], dqs)
                pr_dqs[p] = None
            ot = outp.tile([_C, _N], f32)
            nc.vector.tensor_scalar_add(ot[:, :], cov[:, 0:_N], bias_sb[:, 0:1])
            nc.gpsimd.dma_start(out=out[b], in_=ot[:, :])
    nc.finalize()
    return nc


def _prep_inputs(X, weight, bias):
    import ml_dtypes

    X = np.asarray(X, dtype=np.float32)
    weight = np.asarray(weight, dtype=np.float32)
    bias = np.asarray(bias, dtype=np.float32)

    wtil = (weight - weight.mean(axis=1, keepdims=True)) * (
        np.float32(_WSCALE) / np.sqrt(np.float32(_D))
    )
    # regroup to [w, dq, cb, dd*32+cc], then stack cb pairs into PE
    # row halves: wsel2[p, dq, cp, :] = wsel[p % 64, dq, 2*cp + p//64, :]
    wsel = (
        wtil.reshape(_W, 8, 4, 4, 32)  # w, dq, dd, cb, cc
        .transpose(0, 1, 3, 2, 4)  # w, dq, cb, dd, cc
        .reshape(_W, 8, 4, _C)
    )
    wsel2 = np.empty((2 * _W, 8, 2, _C), dtype=wsel.dtype)
    for cp in range(2):
        wsel2[:_W, :, cp, :] = wsel[:, :, 2 * cp, :]
        wsel2[_W:, :, cp, :] = wsel[:, :, 2 * cp + 1, :]
    wsel = np.ascontiguousarray(wsel2).astype(ml_dtypes.bfloat16)

    # xdt[b, w, j] = X[b, 8j + w] for j < 506 (zero-padded past T)
    Xp = np.zeros((_B, _T + 64), dtype=np.float32)
    Xp[:, :_T] = X
    xd = np.zeros((_B, _W, _JW), dtype=np.float32)
    for w in range(_W):
        xd[:, w, :_MM] = Xp[:, w : w + 8 * _MM : 8]
    xd = xd.astype(ml_dtypes.bfloat16)

    selm = np.zeros((_C, 32), dtype=np.float32)
    for p in range(_C):
        selm[p, p % 32] = 1.0
    selm = selm.astype(ml_dtypes.bfloat16)

    bias2 = np.ascontiguousarray(bias.reshape(_C, 1))

    xd2 = np.concatenate([xd, xd], axis=1)  # duplicate w rows for row tiles

    in_maps = []
    for k in range(_NCORES):
        xdt_k = np.ascontiguousarray(
            xd2[k * _BSH : (k + 1) * _BSH].transpose(1, 0, 2)
        )
        in_maps.append(
            {
                "xdt": xdt_k,
                "wt": wsel,
                "sel": selm,
                "bias": bias2,
            }
        )
    return in_maps


def get_nc():
    global _NC_CACHE
    if _NC_CACHE is None:
        _NC_CACHE = _build_nc()
    return _NC_CACHE


def run(X, weight, bias, trace=False, tmpdir=None):
    """Returns (full_output, BassKernelResults)."""
    from concourse.bass_utils import run_bass_kernel_spmd

    nc = get_nc()
    in_maps = _prep_inputs(X, weight, bias)
    res = run_bass_kernel_spmd(
        nc, in_maps, core_ids=list(range(_NCORES)), trace=trace, tmpdir=tmpdir
    )
    parts = [res.results[i]["out"].transpose(0, 2, 1) for i in range(_NCORES)]
    full = np.ascontiguousarray(np.concatenate(parts, axis=0), dtype=np.float32)
    return full, res


def kernel(X, weight, bias):
    full, _ = run(X, weight, bias)
    return full


# revision 33
# speedup vs baseline: 1.3472x; 1.3472x over previous
"""AutoCov1D Trainium2 kernel (8 NeuronCores, data-parallel over batch).

Math: for window n (stride 8, width 64), with X1 = X[:, :-64], X2 = X[:, 64:]:
  p1 = einsum('bnw,wdc', X1win, Wgt); p2 likewise with X2win
  out = mean_d(p1c * p2c) + bias   (p*c centered over d)

Exact simplifications:
  1. Centering over d is linear in the weight, so pre-center the weight:
     Wtil = (W - mean_d W) / sqrt(D); then no mean terms remain.
  2. X2 windows are X1 windows shifted by 8 window indices (64 = 8*stride),
     so ONE projection P[b,m,:] = sum_w X[b, 8m+w] * Wtil[w,:] over m=0..504
     serves both operands:  out[b,n,c] = sum_d P[b,n,d,c]*P[b,n+8,d,c] + bias.

Implementation notes:
  - X pre-decimated on host: xdt[w, b, j] = X[b, 8j + w] -> contiguous
    projection matmul rhs, 8x less X DMA than a shifted-rows layout; X is
    duplicated into partitions 64-127 so each cb pair of projections runs
    CONCURRENTLY as K=64 PE row tiles at row positions 0 and 64.
  - bf16 products (DVE 2x tensor_tensor, one op per dq pair); latent-dim
    reduction via K=128 -> M=32 selector matmuls at 4 concurrent PE
    column tiles, accumulating the 8 dq quads in fp32 PSUM.
  - Selector matmuls interleave into the same batch row's projection
    stream once their dq pair's products land, avoiding a long serial
    selector tail.
  - PSUM evacuation (fp32->bf16, 1x rate on either engine) splits ~4:1
    ACT:DVE to balance the two engines; DVE additionally runs the
    products and the bias add, ACT only evacuates.
  - PE warmup matmuls on a zeroed scratch tile keep the tensor engine
    busy (p-state) while input DMAs land; DMA triggers are spread across
    the ACT/SP/GPSIMD queues so they issue in parallel at startup.
"""

import sys

import numpy as np

if "/opt/trn_rl_repo" not in sys.path:
    sys.path.insert(0, "/opt/trn_rl_repo")

_B, _T, _W, _D, _C = 32, 4096, 64, 32, 128
_NCORES = 8
_BSH = _B // _NCORES  # 4
_M = 505  # projection windows per batch row
_N = 497  # output windows per batch row
_S = 8  # stride, also window shift in m-space
_MM = 506  # projection matmul free dim (even; last col pad)
_JW = 512  # padded xdt column count
_WSCALE = 1.0  # weight prescale (1 = none)

_NC_CACHE = None


def _build_nc():
    import concourse.bass as bass
    import concourse.tile as tile
    from concourse import bacc, mybir
    from contextlib import ExitStack

    f32 = mybir.dt.float32
    bf16 = mybir.dt.bfloat16

    nc = bacc.Bacc(None, target_bir_lowering=False)
    # xdt[w, b, j] = X[b, 8j + w] (zero past the end), duplicated into
    # rows 64-127 for the row-tiled projection matmuls
    xdt = nc.declare_dram_parameter("xdt", [2 * _W, _BSH, _JW], bf16, isOutput=False)
    # wt[p, dq, cp, dd*32+cc]: p<64 -> Wtil[p, 4*dq+dd, 64*cp+cc+0]
    # row-half layout, p>=64 -> Wtil[p-64, 4*dq+dd, 64*cp+cc+32]
    wt = nc.declare_dram_parameter("wt", [2 * _W, 8, 2, _C], bf16, isOutput=False)
    sel = nc.declare_dram_parameter("sel", [_C, 32], bf16, isOutput=False)
    bias = nc.declare_dram_parameter("bias", [_C, 1], f32, isOutput=False)
    out = nc.declare_dram_parameter("out", [_BSH, _C, _N], f32, isOutput=True)

    with ExitStack() as ctx:
        tc = ctx.enter_context(tile.TileContext(nc))
        singles = ctx.enter_context(tc.tile_pool(name="singles", bufs=1))
        psp = ctx.enter_context(tc.tile_pool(name="psp", bufs=3, space="PSUM"))
        covp = ctx.enter_context(tc.tile_pool(name="covp", bufs=2, space="PSUM"))
        evacp = ctx.enter_context(tc.tile_pool(name="evacp", bufs=4))
        prodp = ctx.enter_context(tc.tile_pool(name="prodp", bufs=4))
        outp = ctx.enter_context(tc.tile_pool(name="outp", bufs=2))

        # PE warmup scratch (zeroed; matmuls on it ramp the p-state while
        # the real input DMAs land)
        warm = singles.tile([_W, _JW], bf16)
        nc.gpsimd.memset(warm[:, :], 0)

        # first dq-slice of weights + first batch row of X land first so
        # real work starts early
        wt_sb = [
            singles.tile([2 * _W, 2, _C], bf16, name=f"wt{dq}") for dq in range(8)
        ]
        # X duplicated into partitions 64-127 so row-tiled projection
        # matmuls at PE row position 64 can stream it
        xdt_sb = [
            singles.tile([2 * _W, _JW], bf16, name=f"xd{b}") for b in range(_BSH)
        ]
        nc.scalar.dma_start(out=xdt_sb[0], in_=xdt[:, 0, :])
        nc.sync.dma_start(out=wt_sb[0], in_=wt[:, 0, :, :])
        sel_sb = singles.tile([_C, 32], bf16)
        nc.gpsimd.dma_start(out=sel_sb, in_=sel[:, :])
        bias_sb = singles.tile([_C, 1], f32)
        nc.gpsimd.dma_start(out=bias_sb, in_=bias[:, :])
        for dq in range(1, 8):
            nc.sync.dma_start(out=wt_sb[dq], in_=wt[:, dq, :, :])
        for b in range(1, _BSH):
            nc.sync.dma_start(out=xdt_sb[b], in_=xdt[:, b, :])

        # warmup matmuls (results discarded)
        wps = covp.tile([_C, _JW], f32, tag="cov")
        for i in range(8):
            nc.tensor.matmul(
                wps[:, 0:_MM],
                lhsT=warm[:, 0:_C],
                rhs=warm[:, 0:_MM],
                start=True,
                stop=True,
            )

        def sel_dq(cov, pr, dq):
            # latent-dim reduction: K=128 -> M=32 selector matmuls, fp32
            # PSUM accumulation over the 8 dq quads per channel block
            for cb in range(4):
                nc.tensor.matmul(
                    cov[32 * cb : 32 * cb + 32, 0:_N],
                    lhsT=sel_sb[:, :],
                    rhs=pr[:, dq % 2, cb, 0:_N],
                    start=(dq == 0),
                    stop=(dq == 7),
                    tile_position=(0, 32 * cb),
                )

        ev_i = 0
        for b in range(_BSH):
            cov = covp.tile([_C, _JW], f32, tag="cov")
            pr_dqs = [None] * 4
            ev = None
            for dq in range(8):
                if dq % 2 == 0:
                    ev = evacp.tile([_C, 2, 4, _JW], bf16, name=f"ev{b}_{dq}", tag="ev")
                    pr_dqs[dq // 2] = prodp.tile(
                        [_C, 2, 4, _JW], bf16, name=f"pr{b}_{dq}", tag="pr"
                    )
                for cp in range(2):
                    ps = psp.tile([_C, 2, 512], f32)
                    # two K=64 projections run concurrently as PE row
                    # tiles at row positions 0 and 64
                    for j in range(2):
                        nc.tensor.matmul(
                            ps[:, j, 0:_MM],
                            lhsT=wt_sb[dq][j * _W : (j + 1) * _W, cp, :],
                            rhs=xdt_sb[b][j * _W : (j + 1) * _W, 0:_MM],
                            start=True,
                            stop=True,
                            tile_position=(j * _W, 0),
                        )
                    # evacuate fp32 PSUM -> bf16 SBUF (1x rate on either
                    # engine; split ~3:1 ACT:DVE to balance engine load)
                    dst = ev[:, dq % 2, 2 * cp : 2 * cp + 2, 0:_MM]
                    if ev_i % 4 == 3 and ev_i % 32 not in (3, 19):
                        nc.vector.tensor_copy(dst, ps[:, :, 0:_MM])
                    else:
                        nc.scalar.copy(out=dst, in_=ps[:, :, 0:_MM])
                    ev_i += 1
                # one shifted-product op per dq pair (8 cb slices)
                if dq % 2 == 1:
                    nc.vector.tensor_mul(
                        pr_dqs[dq // 2][:, :, :, 0:_N],
                        ev[:, :, :, 0:_N],
                        ev[:, :, :, _S : _S + _N],
                    )
                # selector for pair p is ready after product of dq=2p+1;
                # emit with 2-dq slack so the PE never stalls on the
                # evac+product chain
                if dq in (5, 7):
                    p = (dq - 5) // 2
                    for dqs in (2 * p, 2 * p + 1):
                        sel_dq(cov, pr_dqs[p], dqs)
            for p in (2, 3):
                for dqs in (2 * p, 2 * p + 1):
                    sel_dq(cov, pr_dqs[p], dqs)
                pr_dqs[p] = None
            ot = outp.tile([_C, _N], f32)
            nc.vector.tensor_scalar_add(ot[:, :], cov[:, 0:_N], bias_sb[:, 0:1])
            nc.gpsimd.dma_start(out=out[b], in_=ot[:, :])
    nc.finalize()
    return nc


def _prep_inputs(X, weight, bias):
    import ml_dtypes

    X = np.asarray(X, dtype=np.float32)
    weight = np.asarray(weight, dtype=np.float32)
    bias = np.asarray(bias, dtype=np.float32)

    wtil = (weight - weight.mean(axis=1, keepdims=True)) * (
        np.float32(_WSCALE) / np.sqrt(np.float32(_D))
    )
    # regroup to [w, dq, cb, dd*32+cc], then stack cb pairs into PE
    # row halves: wsel2[p, dq, cp, :] = wsel[p % 64, dq, 2*cp + p//64, :]
    wsel = (
        wtil.reshape(_W, 8, 4, 4, 32)  # w, dq, dd, cb, cc
        .transpose(0, 1, 3, 2, 4)  # w, dq, cb, dd, cc
        .reshape(_W, 8, 4, _C)
    )
    wsel2 = np.empty((2 * _W, 8, 2, _C), dtype=wsel.dtype)
    for cp in range(2):
        wsel2[:_W, :, cp, :] = wsel[:, :, 2 * cp, :]
        wsel2[_W:, :, cp, :] = wsel[:, :, 2 * cp + 1, :]
    wsel = np.ascontiguousarray(wsel2).astype(ml_dtypes.bfloat16)

    # xdt[b, w, j] = X[b, 8j + w] for j < 506 (zero-padded past T)
    Xp = np.zeros((_B, _T + 64), dtype=np.float32)
    Xp[:, :_T] = X
    xd = np.zeros((_B, _W, _JW), dtype=np.float32)
    for w in range(_W):
        xd[:, w, :_MM] = Xp[:, w : w + 8 * _MM : 8]
    xd = xd.astype(ml_dtypes.bfloat16)

    selm = np.zeros((_C, 32), dtype=np.float32)
    for p in range(_C):
        selm[p, p % 32] = 1.0
    selm = selm.astype(ml_dtypes.bfloat16)

    bias2 = np.ascontiguousarray(bias.reshape(_C, 1))

    xd2 = np.concatenate([xd, xd], axis=1)  # duplicate w rows for row tiles

    in_maps = []
    for k in range(_NCORES):
        xdt_k = np.ascontiguousarray(
            xd2[k * _BSH : (k + 1) * _BSH].transpose(1, 0, 2)
        )
        in_maps.append(
            {
                "xdt": xdt_k,
                "wt": wsel,
                "sel": selm,
                "bias": bias2,
            }
        )
    return in_maps


def get_nc():
    global _NC_CACHE
    if _NC_CACHE is None:
        _NC_CACHE = _build_nc()
    return _NC_CACHE


def run(X, weight, bias, trace=False, tmpdir=None):
    """Returns (full_output, BassKernelResults)."""
    from concourse.bass_utils import run_bass_kernel_spmd

    nc = get_nc()
    in_maps = _prep_inputs(X, weight, bias)
    res = run_bass_kernel_spmd(
        nc, in_maps, core_ids=list(range(_NCORES)), trace=trace, tmpdir=tmpdir
    )
    parts = [res.results[i]["out"].transpose(0, 2, 1) for i in range(_NCORES)]
    full = np.ascontiguousarray(np.concatenate(parts, axis=0), dtype=np.float32)
    return full, res


def kernel(X, weight, bias):
    full, _ = run(X, weight, bias)
    return full
